# revision 1
# baseline (speedup 1.0000x reference)
"""Trainium2 Bass kernel for nn_CQLoss (composite loss function).

Strategy: pure data parallel over batch dim (64 batches -> 8 per core).
Per core:
  - recon term: rows of [rzs | sqrt(w)*pts] (host-concatenated) gathered by
    `mapping` via indirect DMA straight from HBM — one gather per batch
    fetches both the recon and pts operands; subtract on VectorE, square +
    per-partition accumulate on ScalarE (chunks 0..3) / VectorE (chunk 4, so
    the ScalarE tail ends before the last gather lands).
  - pts term:  pts/pts_gt pre-scaled by sqrt(landmark weight) on the host;
    subtract + square (2x-mode tensor_tensor) + 4x-mode tensor_scalar
    accumulate, all on VectorE.
  - KL term:   ln(V*qy + V*eps) on ScalarE (== ln(qy+eps) - ln(1/V)),
    multiplied by qy (2x) and tensor_scalar-accumulated (4x) on VectorE
    (tensor_reduce is the slowest DVE op - 1x - so it is avoided).
  - best term: tiny; landmark index on the partition dim, host pre-scaled,
    zero-padded to 128 partitions.
The large tensors travel as bf16 (quantization contributes ~5e-5 relative
error on the final scalar; the kernel is HBM-bandwidth-bound so this halves
its runtime). Each core emits per-partition partial sums; the host does the
final (cheap) reduction in float64 and applies the global mean scalings.

Written in raw bass (explicit semaphores): this toolchain's codegen allows at
most one attached sync-wait per compute instruction, so waits are emitted as
standalone wait_ge ops. One semaphore per DMA (increments of concurrent DMAs
on a shared semaphore interleave, so intermediate wait values are racy), and
same-engine back-to-back RAW pairs get an explicit self-wait (engine
pipelines have no interlocks). All constants travel in a single packed DMA
(the int32 mapping rides bit-cast through the f32 pack). All batches are
SBUF-resident; compute is issued in multi-batch chunks with small final
chunks so the end-of-stream serial tail is short.
"""

import os
import sys

import numpy as np

for _p in ("/opt/trn_rl_repo", "/root/.axon_site/_ro/trn_rl_repo"):
    if os.path.isdir(_p) and _p not in sys.path:
        sys.path.insert(0, _p)

B, S, D, P, C, V = 64, 128, 2048, 118, 2, 512
PC = P * C  # 236
K = D + PC  # combined gather row width: 2284
N_CORES = 8
BL = B // N_CORES  # 8 batches per core
ALPHA, BETA, GAMMA, EPS = 10.0, 0.1, 1.0, 1e-20
MARKS = (0, 29, 88, 117)
# disk + ALPHA*landmark == (1/PC) * (sum d^2 + W_MARK * sum_marks d^2) per
# (b,s) row: ALPHA * PC / (len(MARKS)*C) = 10 * 236 / 8
W_MARK = ALPHA * PC / (len(MARKS) * C)  # 295.0

# recon chunking: (start_batch, n_batches) per chunk; small chunks last so the
# end-of-stream gather -> sub -> square chain is short
ZCH = [(0, 2), (2, 2), (4, 2), (6, 1), (7, 1)]

# packed const layout (f32 cols): 0..7 mapping (int32 bits), 8 ln bias,
# 9..24 sqrt(w)*best, 25..40 sqrt(w)*best_gt
NCONST = 9 + 4 * BL * C  # 41

_CACHE: dict = {}


def _build_bass(vector_dims: int):
    import concourse.bass as bass
    from concourse import mybir

    f32 = mybir.dt.float32
    bf16 = mybir.dt.bfloat16
    i32 = mybir.dt.int32
    Act = mybir.ActivationFunctionType
    Alu = mybir.AluOpType

    nc = bass.Bass()

    zs = nc.dram_tensor("zs", [BL * S, D], bf16, kind="ExternalInput")
    # gath rows: [rzs_row (D) | sqrt(w)*pts_row (PC)]
    gath = nc.dram_tensor("gath", [BL * S, K], bf16, kind="ExternalInput")
    ptsgt = nc.dram_tensor("ptsgt", [BL, S, PC], bf16, kind="ExternalInput")
    qy = nc.dram_tensor("qy", [BL, S, V], bf16, kind="ExternalInput")
    cpack = nc.dram_tensor("cpack", [S, NCONST], f32, kind="ExternalInput")
    # partials: cols 0..3 recon chunks 0..3 (ScalarE), col 4 best (ScalarE),
    # col 5 q*log, col 6 pts_h0, col 7 pts_h1, col 8 recon chunk 4 (VectorE)
    po = nc.dram_tensor("po", [S, 9], f32, kind="ExternalOutput")

    ln_scale = float(vector_dims)
    BC = BL * C  # 16

    # DVE op counts:
    #  1 sub_best, 2 mul_q, 3 tsacc_q, 4 sub_rz_c0, 5 sub_rz_c1, 6 sub_rz_c2,
    #  7 sub_pts_h0, 8 sqmul_pts_h0, 9 tsacc_pts_h0, 10 sub_rz_c3,
    #  11 sub_pts_h1, 12 sqmul_pts_h1, 13 tsacc_pts_h1, 14 sub_rz_c4,
    #  15 sqmul_rz_c4, 16 tsacc_rz_c4
    # ACT op counts:
    #  1 sq_best, 2 ln_all, 3..6 sq_rz_c0..c3
    DVE_N = 16
    ACT_N = 6

    from contextlib import ExitStack

    with ExitStack() as ctx:
        zs_t = ctx.enter_context(nc.sbuf_tensor([S, BL * D], bf16))
        gt_t = ctx.enter_context(nc.sbuf_tensor([S, BL * K], bf16))
        qy_t = ctx.enter_context(nc.sbuf_tensor([S, BL * V], bf16))
        lq_t = ctx.enter_context(nc.sbuf_tensor([S, BL * V], bf16))
        pg_t = ctx.enter_context(nc.sbuf_tensor([S, BL * PC], bf16))
        cp_t = ctx.enter_context(nc.sbuf_tensor([S, NCONST], f32))
        bd_t = ctx.enter_context(nc.sbuf_tensor([S, BC], f32))
        acc_t = ctx.enter_context(nc.sbuf_tensor([S, 9], f32))
        sem_cp = ctx.enter_context(nc.semaphore("sem_cp"))
        sem_zs = [
            ctx.enter_context(nc.semaphore(f"sem_zs{c}")) for c in range(len(ZCH))
        ]
        sem_g = [ctx.enter_context(nc.semaphore(f"sem_g{i}")) for i in range(BL)]
        sem_qy = ctx.enter_context(nc.semaphore("sem_qy"))
        sem_pg = ctx.enter_context(nc.semaphore("sem_pg"))
        sem_dve = ctx.enter_context(nc.semaphore("sem_dve"))
        sem_act = ctx.enter_context(nc.semaphore("sem_act"))
        sem_out = ctx.enter_context(nc.semaphore("sem_out"))
        block = ctx.enter_context(nc.Block())

        # 3D views: [s, batch, col]
        gt3 = gt_t[:].rearrange("s (b k) -> s b k", b=BL)
        zs3 = zs_t[:].rearrange("s (b d) -> s b d", b=BL)
        pg3 = pg_t[:].rearrange("s (b p) -> s b p", b=BL)
        map_i = cp_t[:, 0:BL].bitcast(i32)

        @block.sync
        def _(sync):
            sync.dma_start(out=cp_t[:], in_=cpack[:]).then_inc(sem_cp, 16)
            # zs chunk 0 and qy early; ptsgt mid; remaining zs chunks follow
            s0, n0 = ZCH[0]
            sync.dma_start(
                out=zs_t[:, s0 * D : (s0 + n0) * D], in_=zs[s0 * S : (s0 + n0) * S, :]
            ).then_inc(sem_zs[0], 16)
            sync.dma_start(
                out=qy_t[:], in_=qy[:, :, :].rearrange("b s v -> s b v")
            ).then_inc(sem_qy, 16)
            s1, n1 = ZCH[1]
            sync.dma_start(
                out=zs_t[:, s1 * D : (s1 + n1) * D], in_=zs[s1 * S : (s1 + n1) * S, :]
            ).then_inc(sem_zs[1], 16)
            sync.dma_start(
                out=pg_t[:], in_=ptsgt[:, :, :].rearrange("b s p -> s b p")
            ).then_inc(sem_pg, 16)
            # stagger the remaining zs chunks using earlier DMA completions as
            # release clocks, so the shared SDMA engines weave them between
            # the (compute-critical) gathers instead of ahead of all of them
            s2, n2 = ZCH[2]
            sync.wait_ge(sem_zs[0], 16)
            sync.dma_start(
                out=zs_t[:, s2 * D : (s2 + n2) * D], in_=zs[s2 * S : (s2 + n2) * S, :]
            ).then_inc(sem_zs[2], 16)
            s3, n3 = ZCH[3]
            sync.wait_ge(sem_qy, 16)
            sync.dma_start(
                out=zs_t[:, s3 * D : (s3 + n3) * D], in_=zs[s3 * S : (s3 + n3) * S, :]
            ).then_inc(sem_zs[3], 16)
            s4, n4 = ZCH[4]
            sync.wait_ge(sem_zs[1], 16)
            sync.dma_start(
                out=zs_t[:, s4 * D : (s4 + n4) * D], in_=zs[s4 * S : (s4 + n4) * S, :]
            ).then_inc(sem_zs[4], 16)
            sync.wait_ge(sem_act, ACT_N)
            sync.wait_ge(sem_dve, DVE_N)
            sync.dma_start(out=po[:], in_=acc_t[:]).then_inc(sem_out, 16)
            sync.wait_ge(sem_out, 16)

        @block.gpsimd
        def _(gpsimd):
            gpsimd.wait_ge(sem_cp, 16)  # mapping loaded
            for i in range(BL):
                gpsimd.indirect_dma_start(
                    out=gt_t[:, i * K : (i + 1) * K],
                    out_offset=None,
                    in_=gath[:],
                    in_offset=bass.IndirectOffsetOnAxis(
                        ap=map_i[:, i : i + 1], axis=0
                    ),
                ).then_inc(sem_g[i], 16)

        def sub_rz_chunk(c):
            s, n = ZCH[c]
            return nc.vector.tensor_sub(
                gt3[:, s : s + n, :D], gt3[:, s : s + n, :D], zs3[:, s : s + n, :]
            )

        def wait_rz_chunk(vector, c):
            s, n = ZCH[c]
            vector.wait_ge(sem_zs[c], 16)
            for k in range(n):
                vector.wait_ge(sem_g[s + k], 16)

        @block.vector
        def _(vector):
            # best term: bd = sqrt(w)*(best - best_gt)
            vector.wait_ge(sem_cp, 16)
            nc.vector.tensor_sub(
                bd_t[:], cp_t[:, 9 : 9 + BC], cp_t[:, 9 + BC : 9 + 2 * BC]
            ).then_inc(sem_dve, 1)  # 1
            # q-term runs before the first gather-gated sub: it only needs
            # ln_all, so it fills VectorE's early idle window
            vector.wait_ge(sem_act, 2)  # ln_all done
            nc.vector.tensor_mul(lq_t[:], qy_t[:], lq_t[:]).then_inc(sem_dve, 1)  # 2
            vector.wait_ge(sem_dve, 2)  # same-engine RAW: mul_q must retire
            nc.vector.tensor_scalar(
                out=lq_t[:],
                in0=lq_t[:],
                scalar1=1.0,
                scalar2=0.0,
                op0=Alu.mult,
                op1=Alu.add,
                accum_out=acc_t[:, 5:6],
            ).then_inc(sem_dve, 1)  # 3
            wait_rz_chunk(vector, 0)
            sub_rz_chunk(0).then_inc(sem_dve, 1)  # 4
            wait_rz_chunk(vector, 1)
            sub_rz_chunk(1).then_inc(sem_dve, 1)  # 5
            wait_rz_chunk(vector, 2)
            sub_rz_chunk(2).then_inc(sem_dve, 1)  # 6
            # pts half 0: d = xm - gt (in place), pg = d*d, 4x accum
            for i in range(4):
                vector.wait_ge(sem_g[i], 16)
            vector.wait_ge(sem_pg, 16)
            nc.vector.tensor_sub(
                gt3[:, 0:4, D:], gt3[:, 0:4, D:], pg3[:, 0:4, :]
            ).then_inc(sem_dve, 1)  # 7
            vector.wait_ge(sem_dve, 7)
            nc.vector.tensor_mul(
                pg3[:, 0:4, :], gt3[:, 0:4, D:], gt3[:, 0:4, D:]
            ).then_inc(sem_dve, 1)  # 8
            vector.wait_ge(sem_dve, 8)
            nc.vector.tensor_scalar(
                out=pg_t[:, : 4 * PC],
                in0=pg_t[:, : 4 * PC],
                scalar1=1.0,
                scalar2=0.0,
                op0=Alu.mult,
                op1=Alu.add,
                accum_out=acc_t[:, 6:7],
            ).then_inc(sem_dve, 1)  # 9
            wait_rz_chunk(vector, 3)
            sub_rz_chunk(3).then_inc(sem_dve, 1)  # 10
            # pts half 1 runs while the last zs chunk's DMA is in flight
            for i in range(4, 8):
                vector.wait_ge(sem_g[i], 16)
            nc.vector.tensor_sub(
                gt3[:, 4:8, D:], gt3[:, 4:8, D:], pg3[:, 4:8, :]
            ).then_inc(sem_dve, 1)  # 11
            vector.wait_ge(sem_dve, 11)
            nc.vector.tensor_mul(
                pg3[:, 4:8, :], gt3[:, 4:8, D:], gt3[:, 4:8, D:]
            ).then_inc(sem_dve, 1)  # 12
            vector.wait_ge(sem_dve, 12)
            nc.vector.tensor_scalar(
                out=pg_t[:, 4 * PC :],
                in0=pg_t[:, 4 * PC :],
                scalar1=1.0,
                scalar2=0.0,
                op0=Alu.mult,
                op1=Alu.add,
                accum_out=acc_t[:, 7:8],
            ).then_inc(sem_dve, 1)  # 13
            # recon chunk 4 squared on DVE (d^2 lands in the consumed zs
            # batch-7 slot)
            wait_rz_chunk(vector, 4)
            sub_rz_chunk(4).then_inc(sem_dve, 1)  # 14
            s4 = ZCH[4][0]
            vector.wait_ge(sem_dve, 14)
            nc.vector.tensor_mul(
                zs3[:, s4, :], gt3[:, s4, :D], gt3[:, s4, :D]
            ).then_inc(sem_dve, 1)  # 15
            vector.wait_ge(sem_dve, 15)
            nc.vector.tensor_scalar(
                out=zs3[:, s4, :],
                in0=zs3[:, s4, :],
                scalar1=1.0,
                scalar2=0.0,
                op0=Alu.mult,
                op1=Alu.add,
                accum_out=acc_t[:, 8:9],
            ).then_inc(sem_dve, 1)  # 16

        @block.scalar
        def _(scalar):
            # best term: acc_t[:, 4] = per-partition sum(bd^2)
            scalar.wait_ge(sem_dve, 1)
            nc.scalar.activation(
                bd_t[:], bd_t[:], Act.Square, accum_out=acc_t[:, 4:5]
            ).then_inc(sem_act, 1)  # 1
            scalar.wait_ge(sem_qy, 16)
            nc.scalar.activation(
                lq_t[:], qy_t[:], Act.Ln, bias=cp_t[:, 8:9], scale=ln_scale
            ).then_inc(sem_act, 1)  # 2
            dve_at = {0: 4, 1: 5, 2: 6, 3: 10}
            for c in range(4):
                s, n = ZCH[c]
                scalar.wait_ge(sem_dve, dve_at[c])
                nc.scalar.activation(
                    gt3[:, s : s + n, :D],
                    gt3[:, s : s + n, :D],
                    Act.Square,
                    accum_out=acc_t[:, c : c + 1],
                ).then_inc(sem_act, 1)  # 3..6

    return nc


def _get_nc(vector_dims: int):
    key = ("nc", vector_dims)
    if key not in _CACHE:
        _CACHE[key] = _build_bass(vector_dims)
    return _CACHE[key]


def _prepare(inputs):
    import ml_dtypes

    bf16 = ml_dtypes.bfloat16

    zs = np.asarray(inputs["zs"], dtype=np.float32)
    rzs = np.asarray(inputs["rzs"], dtype=np.float32)
    pts = np.asarray(inputs["pts"], dtype=np.float32)
    pts_gt = np.asarray(inputs["pts_gt"], dtype=np.float32)
    qy = np.asarray(inputs["qy"], dtype=np.float32)
    best = np.asarray(inputs["best"], dtype=np.float64)
    best_gt = np.asarray(inputs["best_gt"], dtype=np.float64)
    mapping = np.asarray(inputs["mapping"])
    vector_dims = int(np.asarray(inputs["vector_dims"]))

    # sqrt of landmark weights, applied on the host (exact in f64)
    w_p = np.ones(P, dtype=np.float64)
    w_p[list(MARKS)] += W_MARK
    w_sq = np.sqrt(w_p)  # (118,)
    wc = w_sq[None, None, :, None]  # broadcast over (B, S, P, C)

    zs_b = np.ascontiguousarray(zs.astype(bf16))
    qy_b = np.ascontiguousarray(qy.astype(bf16))
    ptsgt_b = np.ascontiguousarray((pts_gt * wc).astype(bf16))
    # combined gather source: [rzs | sqrt(w)*pts] per row
    gath_b = np.empty((B, S, K), dtype=bf16)
    gath_b[:, :, :D] = rzs.astype(bf16)
    gath_b[:, :, D:] = (pts * wc).astype(bf16).reshape(B, S, PC)
    best_w = (best * w_sq[None, :, None]).astype(np.float32)
    bestgt_w = (best_gt * w_sq[None, :, None]).astype(np.float32)

    base = (np.arange(BL, dtype=np.int32) * S)[:, None]  # absolute row offsets
    BC = BL * C

    in_maps = []
    for c in range(N_CORES):
        sl = slice(c * BL, (c + 1) * BL)
        map_abs = np.ascontiguousarray(
            (mapping[sl].astype(np.int32) + base).T
        )  # (S, BL)
        cpk = np.zeros((S, NCONST), dtype=np.float32)
        cpk[:, 0:BL] = map_abs.view(np.float32)
        cpk[:, BL] = np.float32(vector_dims * EPS)
        cpk[:P, 9 : 9 + BC] = best_w[sl].transpose(1, 0, 2).reshape(P, BC)
        cpk[:P, 9 + BC : 9 + 2 * BC] = bestgt_w[sl].transpose(1, 0, 2).reshape(P, BC)
        in_maps.append(
            {
                "zs": zs_b[sl].reshape(BL * S, D),
                "gath": gath_b[sl].reshape(BL * S, K),
                "ptsgt": ptsgt_b[sl].reshape(BL, S, PC),
                "qy": qy_b[sl],
                "cpack": cpk,
            }
        )
    return in_maps, vector_dims


def _combine(results) -> np.ndarray:
    s_pts = np.float64(0.0)
    s_kl = np.float64(0.0)
    s_best = np.float64(0.0)
    s_recon = np.float64(0.0)
    for r in results:
        por = r["po"].astype(np.float64)
        s_recon += por[:, 0:4].sum() + por[:, 8].sum()
        s_best += por[:, 4].sum()
        s_kl += por[:, 5].sum()
        s_pts += por[:, 6:8].sum()

    kld = s_kl / (B * S)
    recon = s_recon / (B * S * D)
    pts_term = s_pts / (B * S * PC)
    best_term = s_best / (B * PC)
    total = BETA * kld + GAMMA * recon + pts_term + best_term
    return np.float32(total)


def kernel(**inputs) -> np.ndarray:
    from concourse.bass_utils import run_bass_kernel_spmd

    in_maps, vector_dims = _prepare(inputs)
    nc = _get_nc(vector_dims)

    trace = os.environ.get("KERNEL_TRACE", "") == "1"
    res = run_bass_kernel_spmd(nc, in_maps, core_ids=list(range(N_CORES)), trace=trace)
    if trace and res.exec_time_ns is not None:
        print(f"HW exec time: {res.exec_time_ns} ns")
        if res.instructions_and_trace is not None:
            print(f"trace: {res.instructions_and_trace[1]}")

    return _combine(res.results)

